# revision 8
# baseline (speedup 1.0000x reference)
"""DGLJTNNDecoder kernel for 8x Trainium2 NeuronCores (Bass/Tile), v2.

Tree-GRU decoder over B=512 chain-trees (N=48 nodes), T=94 DFS steps,
followed by two MLP heads producing (q_loss, p_loss, q_acc, p_acc).

v2 structure (vs v1):
  - embedding gather + transpose moved to the host (x^T arrives via DMA)
  - scan uses sigmoid-only activations: tanh(a) = 2*sigmoid(2a) - 1 with
    the factor folded into host-scaled weights (state kept as m-hat = m/2)
  - per-node projections A_z/A_h2/A_r are re-injected into PSUM via
    identity matmuls (PE) instead of DVE adds, so the sigmoid reads the
    fully-accumulated argument straight from PSUM
  - both DFS chains (forward + backtrack) run lockstep in one instruction
    stream: every matmul covers both chains (F=128)
  - exp/ln activations grouped to avoid ACT table-set reloads
  - q-head target logit/argmax via exp-space STT + pairwise reduce_max
"""

import sys

if "/opt/trn_rl_repo" not in sys.path:
    sys.path.insert(0, "/opt/trn_rl_repo")

import numpy as np

B, N, H, L, V = 512, 48, 256, 64, 800
NC = 8
BC = B // NC            # 64 trees per core
NF = N - 1              # 47 steps per chain
NODES = N * BC          # 3072 node columns per core
QBLK = NF + 1           # 48
PBLK = 2 * NF + 1       # 95
PROWS = PBLK * BC       # 6080
PPAD = 48 * 128         # 6144

_CACHE = {}


def _build(wob_nonzero: bool):
    import concourse.bass as bass
    import concourse.tile as tile
    from concourse import bacc, mybir
    from concourse.masks import make_identity

    f32 = mybir.dt.float32
    i32 = mybir.dt.int32
    wdt = mybir.dt.bfloat16
    AF = mybir.ActivationFunctionType
    ALU = mybir.AluOpType
    AX = mybir.AxisListType

    nc = bacc.Bacc()

    def din(name, shape, dtype=f32):
        return nc.declare_dram_parameter(name, list(shape), dtype, isOutput=False)

    # --- DRAM parameters ------------------------------------------------
    xt_d = din("xt", [H, NODES], wdt)        # x^T node-major
    xtr_d = din("xtr", [H, NODES], wdt)      # x_v^T for backtrack (padded)
    tvt = din("tvt", [L, 8 * BC], wdt)
    qtgt = din("qtgt", [128, 24])
    ptgt = din("ptgt", [128, 48])
    WzT = din("WzT", [H, H], wdt)            # [K,M] halves stacked: rearr2
    WhT2 = din("WhT2", [H, H], wdt)
    WrW = din("WrW", [H, H], wdt)
    WzB2 = din("WzB2", [H, H], wdt)
    WhB4 = din("WhB4", [H, H], wdt)
    Ur2 = din("Ur2", [H, H], wdt)
    UwX = din("UwX", [H, H], wdt)
    UwH2 = din("UwH2", [H, H], wdt)
    UwL = din("UwL", [L, H], wdt)
    WwH2 = din("WwH2", [H, H], wdt)
    WwL = din("WwL", [L, H], wdt)
    Wo = din("Wo", [H, V], wdt)
    Us = din("Us", [H, 1], wdt)
    bz2 = din("bz2", [128, 2]); bh22 = din("bh22", [128, 2]); br2 = din("br2", [128, 2])
    ub2 = din("ub2", [128, 2]); wb2 = din("wb2", [128, 2])
    usb = din("usb", [128, 1])
    wob = din("wob", [1, V]) if wob_nonzero else None
    outp = nc.declare_dram_parameter("outp", [128, 8], f32, isOutput=True)

    def rearr2(ap):
        return ap.rearrange("(k p) m -> p k m", p=128)

    with tile.TileContext(nc) as tc:
        with (
            tc.tile_pool(name="persist", bufs=1) as pp,
            tc.tile_pool(name="small", bufs=1) as sp,
        ):
            # --- load weights/constants ---------------------------------
            def loadw(dram, shape, tag, dt=wdt, re2=True, eng=None):
                t = pp.tile(shape, dt, tag=tag)
                e = eng or nc.sync
                e.dma_start(out=t, in_=rearr2(dram[:]) if re2 else dram[:])
                return t

            # phase-B-critical first
            xt = loadw(xt_d, [128, 2, NODES], "xt")
            wzt_s = loadw(WzT, [128, 2, H], "wzt")
            wht2_s = loadw(WhT2, [128, 2, H], "wht2")
            wr_s = loadw(WrW, [128, 2, H], "wr")
            wzb2_s = loadw(WzB2, [128, 2, H], "wzb2")
            whb4_s = loadw(WhB4, [128, 2, H], "whb4")
            ur2_s = loadw(Ur2, [128, 2, H], "ur2")
            xtr = loadw(xtr_d, [128, 2, NODES], "xtr", eng=nc.gpsimd)
            uwx_s = loadw(UwX, [128, 2, H], "uwx", eng=nc.gpsimd)
            uwh2_s = loadw(UwH2, [128, 2, H], "uwh2", eng=nc.gpsimd)
            wwh2_s = loadw(WwH2, [128, 2, H], "wwh2", eng=nc.gpsimd)
            wo_s = loadw(Wo, [128, 2, V], "wo", eng=nc.gpsimd)
            us_s = loadw(Us, [128, 2, 1], "us", eng=nc.gpsimd)
            uwl_s = loadw(UwL, [L, H], "uwl", re2=False, eng=nc.gpsimd)
            wwl_s = loadw(WwL, [L, H], "wwl", re2=False, eng=nc.gpsimd)
            bz_s = loadw(bz2, [128, 2], "bz", dt=f32, re2=False, eng=nc.gpsimd)
            bh2_s = loadw(bh22, [128, 2], "bh2", dt=f32, re2=False, eng=nc.gpsimd)
            br_s = loadw(br2, [128, 2], "br", dt=f32, re2=False, eng=nc.gpsimd)
            ub_s = loadw(ub2, [128, 2], "ub", dt=f32, re2=False, eng=nc.gpsimd)
            wb_s = loadw(wb2, [128, 2], "wb", dt=f32, re2=False, eng=nc.gpsimd)
            usb_s = loadw(usb, [128, 1], "usb", dt=f32, re2=False, eng=nc.gpsimd)
            qtgt_s = loadw(qtgt, [128, 24], "qtgt", dt=f32, re2=False, eng=nc.gpsimd)
            ptgt_s = loadw(ptgt, [128, 48], "ptgt", dt=f32, re2=False, eng=nc.gpsimd)
            wob_s = (
                loadw(wob, [1, V], "wob", dt=f32, re2=False, eng=nc.gpsimd)
                if wob_nonzero else None
            )
            tvrep = pp.tile([L, 8, BC], wdt, tag="tvrep")
            nc.gpsimd.dma_start(
                out=tvrep, in_=tvt[:].rearrange("l (r b) -> l r b", b=BC)
            )

            ident = pp.tile([128, 128], wdt, tag="ident")
            make_identity(nc, ident)

            iota_f = pp.tile([128, V], mybir.dt.float16, tag="iotaf")
            with tc.tile_pool(name="iota", bufs=1) as ip:
                iota_i = ip.tile([128, V], i32, tag="iotai")
                nc.gpsimd.iota(iota_i, pattern=[[1, V]], base=0, channel_multiplier=0)
                nc.vector.tensor_copy(iota_f, iota_i)

            # arenas + scan state
            # A3: [mt, gate(z, h2, r), node] - one fold matmul per (mt, c)
            # covers all three gates (src(t) == dst(t-1) on a chain)
            a3 = pp.tile([128, 2, 3, NODES], wdt, tag="a3")
            # MH: slot-major m-hat history: [slot, kt, c, b]
            MH = pp.tile([128, NF + 2, 2, 2, BC], wdt, tag="mh")
            rm_t = [
                pp.tile([128, 2, 2, BC], wdt, tag=f"rm{i}", name=f"rm{i}")
                for i in range(2)
            ]
            zrm = pp.tile([128, 2, 2, BC], wdt, tag="zrm")
            nc.vector.memset(MH[:, 0], 0.0)
            nc.vector.memset(MH[:, NF + 1], 0.0)
            nc.vector.memset(zrm, 0.0)

            outp_s = sp.tile([128, 8], f32, tag="outp")
            nc.vector.memset(outp_s, 0.0)
            sume = sp.tile([128, 24], f32, tag="sume")
            qt_exp = sp.tile([128, 24], f32, tag="qte")
            rmax = sp.tile([128, 24], wdt, tag="rmax")

            # --- Phase B: A_z / A_h2 / A_r (emitted in tranches) --------
            cp_engines = [nc.scalar, nc.vector]
            pb_idx = [0]

            def pb_job(pool, ch, gate, mt):
                csl = slice(ch * 512, (ch + 1) * 512)
                msl = slice(mt * 128, (mt + 1) * 128)
                w_s = (wzt_s, wht2_s, wr_s)[gate]
                b_s = (bz_s, bh2_s, br_s)[gate]
                ps = pool.tile([128, 512], f32, tag="ph", name="ph")
                for kt in range(2):
                    nc.tensor.matmul(
                        ps, w_s[:, kt, msl], xt[:, kt, csl],
                        start=(kt == 0), stop=(kt == 1),
                    )
                dst = a3[:, mt, gate, csl]
                eng = cp_engines[pb_idx[0] % 2]; pb_idx[0] += 1
                if eng is nc.scalar:
                    nc.scalar.activation(
                        dst, ps, AF.Identity, bias=b_s[:, mt : mt + 1]
                    )
                else:
                    eng.tensor_scalar(
                        out=dst, in0=ps, scalar1=b_s[:, mt : mt + 1],
                        scalar2=None, op0=ALU.add,
                    )

            def pb_tranche(pool, chs):
                for ch in chs:
                    for gate in range(3):
                        for mt in range(2):
                            pb_job(pool, ch, gate, mt)

            # --- Scan + interleaved head projections --------------------
            # P1[u] accumulates: z/h args of step u (g0,g1) and the r arg
            # of step u-1 (g2).  Folds+r-rec are issued one step ahead so
            # the PE never queues behind an un-ready dependency.
            p1f = pp.tile([128, 2, NODES], wdt, tag="p1f")
            p1b = pp.tile([128, 2, NODES], wdt, tag="p1b")
            q1 = pp.tile([128, 2, NODES], wdt, tag="q1")

            def relu_copy(eng, dst, ps, b_s, mt):
                if eng is nc.scalar:
                    nc.scalar.activation(dst, ps, AF.Relu, bias=b_s[:, mt : mt + 1])
                else:
                    eng.tensor_scalar(
                        out=dst, in0=ps, scalar1=b_s[:, mt : mt + 1],
                        scalar2=0.0, op0=ALU.add, op1=ALU.max,
                    )

            # head-projection chunk jobs: (kind, ch, mt).  ready = first scan
            # iteration at which every MH slot the job reads exists.
            def hjob(php_pool, kind, ch, mt, eng):
                csl = slice(ch * 512, (ch + 1) * 512)
                msl = slice(mt * 128, (mt + 1) * 128)
                ps = php_pool.tile([128, 512], f32, tag="ph")
                if kind == "q":
                    for kt in range(2):
                        nc.tensor.matmul(
                            ps, wwh2_s[:, kt, msl],
                            MH[:, ch * 8 : (ch + 1) * 8, kt, 0, :],
                            start=(kt == 0), stop=False,
                        )
                    nc.tensor.matmul(
                        ps, wwl_s[:, msl], tvrep[:, :8, :], start=False, stop=True
                    )
                    relu_copy(eng, q1[:, mt, csl], ps, wb_s, mt)
                    return
                xsrc = xt if kind == "pf" else xtr
                hs = (
                    MH[:, ch * 8 : (ch + 1) * 8, 0, 0, :],
                    MH[:, ch * 8 : (ch + 1) * 8, 1, 0, :],
                ) if kind == "pf" else (
                    MH[:, 1 + ch * 8 : 1 + (ch + 1) * 8, 0, 1, :],
                    MH[:, 1 + ch * 8 : 1 + (ch + 1) * 8, 1, 1, :],
                )
                for kt in range(2):
                    nc.tensor.matmul(
                        ps, uwx_s[:, kt, msl], xsrc[:, kt, csl],
                        start=(kt == 0), stop=False,
                    )
                for kt in range(2):
                    nc.tensor.matmul(
                        ps, uwh2_s[:, kt, msl], hs[kt], start=False, stop=False
                    )
                nc.tensor.matmul(
                    ps, uwl_s[:, msl], tvrep[:, :8, :], start=False, stop=True
                )
                dstp = (p1f if kind == "pf" else p1b)[:, mt, csl]
                relu_copy(eng, dstp, ps, ub_s, mt)

            # in-scan schedule tables
            fill_jobs = {}   # iter -> list of (kind, ch, mt)
            for ch in range(5):
                for off, job in enumerate(
                    [("pf", ch, 0), ("pf", ch, 1), ("q", ch, 0), ("q", ch, 1)]
                ):
                    fill_jobs.setdefault(8 * ch + 8 + off, []).append(job)
            for off, job in enumerate(
                [("pb", 2, 0), ("pb", 2, 1), ("pb", 3, 0), ("pb", 3, 1)]
            ):
                fill_jobs.setdefault(31 + off, []).append(job)
            for off, job in enumerate([("pb", 4, 0), ("pb", 4, 1)]):
                fill_jobs.setdefault(42 + off, []).append(job)
            hb_at = {}       # iter -> list of k
            for k in range(NF - 1):
                hb_at.setdefault(max(k, NF - 2 - k) + 1, []).append(k)

            def q2_job(qpool, j):
                psq = qpool.tile([128, V], f32, tag="qlg", name="qlg")
                for kt in range(2):
                    for n0, nn in ((0, 512), (512, V - 512)):
                        nc.tensor.matmul(
                            psq[:, n0 : n0 + nn],
                            q1[:, kt, j * 128 : (j + 1) * 128],
                            wo_s[:, kt, n0 : n0 + nn],
                            start=(kt == 0), stop=(kt == 1),
                        )
                if wob_nonzero:
                    wv = wob_s[:]
                    wb_b = bass.AP(
                        tensor=wv.tensor, offset=wv.offset,
                        ap=[[0, 128], [1, V]],
                    )
                    nc.vector.tensor_add(psq, psq, wb_b)
                return psq

            def hb_add(k):
                j = NF - 2 - k
                nc.gpsimd.tensor_add(
                    MH[:, k + 1, :, 1, :], MH[:, k + 1, :, 1, :],
                    MH[:, j + 1, :, 0, :],
                )

            with (
                tc.tile_pool(name="zp", bufs=1, space="PSUM") as zp,
                tc.tile_pool(name="hp", bufs=2, space="PSUM") as hp,
                tc.tile_pool(name="rp", bufs=1, space="PSUM") as rp,
                tc.tile_pool(name="php", bufs=2, space="PSUM") as php,
                tc.tile_pool(name="sst", bufs=3) as st,
            ):
                pb_tranche(php, (5, 0))
                pb_rest = [
                    (ch, gate, mt)
                    for ch in (4, 1, 3, 2)
                    for gate in range(3)
                    for mt in range(2)
                ]
                for t in range(NF):
                    for _ in range(2):
                        if pb_rest:
                            pb_job(php, *pb_rest.pop(0))
                    nsrc = (t, NF - t)
                    ndst = (t + 1, NF - 1 - t)
                    rmp = zrm if t == 0 else rm_t[(t - 1) % 2]

                    # z gate: own tile, sigma_z releases as soon as z-rec lands
                    Pz = zp.tile([128, 2, 2, 2, BC], f32, tag="pz", name="pz")
                    for mt in range(2):
                        for c in range(2):
                            nc.tensor.matmul(
                                Pz[:, mt, c, 0, :], ident,
                                a3[:, mt, 0, nsrc[c] * BC : (nsrc[c] + 1) * BC],
                                start=(mt == 0 and c == 0), stop=False,
                            )
                    for mt in range(2):
                        msl = slice(mt * 128, (mt + 1) * 128)
                        for kt in range(2):
                            nc.tensor.matmul(
                                Pz[:, mt, :, 0, :], wzb2_s[:, kt, msl],
                                MH[:, t, kt, :, :],
                                start=False, stop=(mt == 1 and kt == 1),
                            )
                    # h gate: own tile so sigma_h's group-stop is h-rec
                    Ph = hp.tile([128, 2, 2, 2, BC], f32, tag="ph1", name="ph1")
                    for mt in range(2):
                        for c in range(2):
                            nc.tensor.matmul(
                                Ph[:, mt, c, 0, :], ident,
                                a3[:, mt, 1, nsrc[c] * BC : (nsrc[c] + 1) * BC],
                                start=(mt == 0 and c == 0), stop=False,
                            )
                    for mt in range(2):
                        msl = slice(mt * 128, (mt + 1) * 128)
                        for kt in range(2):
                            nc.tensor.matmul(
                                Ph[:, mt, :, 0, :], whb4_s[:, kt, msl],
                                rmp[:, kt, :, :],
                                start=False, stop=(mt == 1 and kt == 1),
                            )
                    zb = st.tile([128, 2, 2, BC], f32, tag="zb")
                    nc.scalar.activation(zb, Pz[:, :, :, 0, :], AF.Sigmoid)
                    sb = st.tile([128, 2, 2, BC], f32, tag="sb")
                    nc.scalar.activation(sb, Ph[:, :, :, 0, :], AF.Sigmoid)
                    # vn = (z-1) * mhat_prev  (ready early, off critical path)
                    vn = st.tile([128, 2, 2, BC], f32, tag="vn")
                    nc.vector.scalar_tensor_tensor(
                        out=vn, in0=zb, scalar=1.0, in1=MH[:, t, :, :, :],
                        op0=ALU.subtract, op1=ALU.mult,
                    )
                    uh = st.tile([128, 2, 2, BC], f32, tag="uh")
                    nc.vector.scalar_tensor_tensor(
                        out=uh, in0=sb, scalar=0.5, in1=zb,
                        op0=ALU.subtract, op1=ALU.mult,
                    )
                    nc.vector.tensor_sub(MH[:, t + 1, :, :, :], uh, vn)

                    if t < NF - 1:
                        Pr = rp.tile([128, 2, 2, 2, BC], f32, tag="pr", name="pr")
                        for mt in range(2):
                            for c in range(2):
                                nc.tensor.matmul(
                                    Pr[:, mt, c, 0, :], ident,
                                    a3[:, mt, 2, ndst[c] * BC : (ndst[c] + 1) * BC],
                                    start=(mt == 0 and c == 0), stop=False,
                                )
                        for mt in range(2):
                            msl = slice(mt * 128, (mt + 1) * 128)
                            for kt in range(2):
                                nc.tensor.matmul(
                                    Pr[:, mt, :, 0, :], ur2_s[:, kt, msl],
                                    MH[:, t + 1, kt, :, :],
                                    start=False, stop=(mt == 1 and kt == 1),
                                )
                        rb = st.tile([128, 2, 2, BC], f32, tag="rb")
                        nc.scalar.activation(rb, Pr[:, :, :, 0, :], AF.Sigmoid)
                        nc.gpsimd.tensor_mul(
                            rm_t[t % 2], rb, MH[:, t + 1, :, :, :]
                        )

                    for k in hb_at.get(t, ()):
                        hb_add(k)
                    for kind, ch, mt in fill_jobs.get(t, ()):
                        hjob(php, kind, ch, mt, nc.vector)


                # post-scan leftovers
                for t in sorted(k for k in hb_at if k >= NF):
                    for k in hb_at[t]:
                        hb_add(k)
                ridx = 0
                for kind, ch in (
                    ("pf", 5), ("q", 5), ("pb", 0), ("pb", 1), ("pb", 5)
                ):
                    for mt in range(2):
                        hjob(php, kind, ch, mt, cp_engines[ridx % 2]); ridx += 1

            # pad rows of p1b -> 0 so pad p-score = Us_b
            nc.vector.memset(p1b[:, :, NF * BC :], 0.0)

            # p2: 48 score columns
            with tc.tile_pool(name="p2p", bufs=1, space="PSUM") as p2p:
                psp = p2p.tile([128, 512], f32, tag="psp")
                for j in range(48):
                    srcp = p1f if j < 24 else p1b
                    jj = j if j < 24 else j - 24
                    for kt in range(2):
                        nc.tensor.matmul(
                            psp[:, j : j + 1],
                            srcp[:, kt, jj * 128 : (jj + 1) * 128],
                            us_s[:, kt, :],
                            start=(kt == 0), stop=(kt == 1),
                        )
                p_sb = sp.tile([128, 48], f32, tag="psb")
                nc.scalar.activation(p_sb, psp[:, :48], AF.Identity, bias=usb_s[:, 0:1])

            # BCE pieces (exp now, ln later with the q-head lns)
            ab_t = sp.tile([128, 48], f32, tag="abt")
            nc.scalar.activation(ab_t, p_sb, AF.Abs)
            en_t = sp.tile([128, 48], f32, tag="ent")
            nc.scalar.activation(en_t, ab_t, AF.Exp, scale=-1.0)
            rl_t = sp.tile([128, 48], f32, tag="rlt")
            nc.gpsimd.tensor_scalar(
                out=rl_t, in0=p_sb, scalar1=0.0, scalar2=None, op0=ALU.max
            )
            ptt = sp.tile([128, 48], f32, tag="ptt")
            nc.gpsimd.tensor_mul(ptt, p_sb, ptgt_s)
            gtz = sp.tile([128, 48], f32, tag="gtz")
            nc.vector.tensor_scalar(
                out=gtz, in0=p_sb, scalar1=0.0, scalar2=None, op0=ALU.is_gt
            )
            pcr = sp.tile([128, 48], f32, tag="pcr")
            nc.vector.tensor_tensor(out=pcr, in0=gtz, in1=ptgt_s, op=ALU.is_equal)
            nc.vector.reduce_sum(outp_s[:, 1:2], pcr, axis=AX.X)

            # --- q-head: logits -> exp-space reductions -----------------
            with (
                tc.tile_pool(name="qps", bufs=2, space="PSUM") as qps,
                tc.tile_pool(name="qsc", bufs=2) as qsc,
            ):
                scr2 = pp.tile([128, V], wdt, tag="scr2")
                scrt = None
                for j in range(24):
                    if j % 2 == 0:
                        scrt = qsc.tile([128, 2, V], wdt, tag="scr", name="scr")
                    psq = q2_job(qps, j)
                    nc.scalar.activation(
                        scrt[:, j % 2, :], psq, AF.Exp,
                        accum_out=sume[:, j : j + 1],
                    )
                    nc.vector.scalar_tensor_tensor(
                        out=scr2, in0=iota_f, scalar=qtgt_s[:, j : j + 1],
                        in1=scrt[:, j % 2, :], op0=ALU.is_equal, op1=ALU.mult,
                        accum_out=qt_exp[:, j : j + 1],
                    )
                    if j % 2 == 1:
                        nc.vector.tensor_reduce(
                            rmax[:, j - 1 : j + 1], scrt, axis=AX.X, op=ALU.max
                        )

            # --- ln era + final reductions ------------------------------
            l1p = sp.tile([128, 48], f32, tag="l1p")
            nc.scalar.activation(l1p, en_t, AF.Ln, bias=1.0)
            lnq = sp.tile([128, 24], f32, tag="lnq")
            nc.scalar.activation(lnq, sume, AF.Ln)
            lnt = sp.tile([128, 24], f32, tag="lnt")
            nc.scalar.activation(lnt, qt_exp, AF.Ln)

            sp_t = sp.tile([128, 48], f32, tag="spt")
            nc.vector.tensor_add(sp_t, l1p, rl_t)
            bce = sp.tile([128, 48], f32, tag="bce")
            nc.vector.tensor_sub(bce, sp_t, ptt)
            nc.vector.reduce_sum(outp_s[:, 0:1], bce, axis=AX.X)
            nc.vector.reduce_sum(outp_s[:, 2:3], lnq, axis=AX.X)
            nc.vector.reduce_sum(outp_s[:, 3:4], lnt, axis=AX.X)
            qc = sp.tile([128, 24], f32, tag="qc")
            nc.vector.tensor_tensor(out=qc, in0=qt_exp, in1=rmax, op=ALU.is_ge)
            nc.vector.reduce_sum(outp_s[:, 4:5], qc, axis=AX.X)
            nc.sync.dma_start(out=outp[:], in_=outp_s)

    nc.finalize()
    return nc


def _get_nc(wob_nonzero: bool):
    key = ("nc", wob_nonzero)
    if key not in _CACHE:
        _CACHE[key] = _build(wob_nonzero)
    return _CACHE[key]


def _wdt_np():
    import ml_dtypes

    return ml_dtypes.bfloat16


def _prep_inputs(inputs):
    f = lambda k: np.ascontiguousarray(np.asarray(inputs[k]), dtype=np.float32)
    wdt = _wdt_np()
    w = lambda a: np.ascontiguousarray(a).astype(wdt)
    wid = np.asarray(inputs["wid"]).astype(np.int64).reshape(B, N)
    tree_vec = f("tree_vec")
    Wz, bz = f("Wz"), f("bz")
    Wr_, Ur_, br = f("Wr"), f("Ur"), f("br")
    Wh, bh = f("Wh"), f("bh")
    W_w, W_b = f("W_w"), f("W_b")
    U_w, U_b = f("U_w"), f("U_b")
    Wo_w, Wo_b = f("Wo_w"), f("Wo_b")
    Us_w, Us_b = f("Us_w"), f("Us_b")
    emb = f("embedding")

    x_full = emb[wid]  # [B, N, H] host-side gather

    def c2(v):
        return np.ascontiguousarray(v.reshape(2, 128).T)

    shared = dict(
        WzT=w(Wz[:H]), WhT2=w(2 * Wh[:H]), WrW=w(Wr_),
        WzB2=w(2 * Wz[H:]), WhB4=w(4 * Wh[H:]), Ur2=w(2 * Ur_),
        UwX=w(U_w[:H]), UwH2=w(2 * U_w[H : 2 * H]), UwL=w(U_w[2 * H :]),
        WwH2=w(2 * W_w[:H]), WwL=w(W_w[H:]),
        Wo=w(Wo_w), Us=w(Us_w),
        bz2=c2(bz), bh22=c2(2 * bh), br2=c2(br), ub2=c2(U_b), wb2=c2(W_b),
        usb=np.full((128, 1), float(Us_b.reshape(-1)[0]), np.float32),
    )
    wob_nonzero = bool(np.any(Wo_b != 0))
    if wob_nonzero:
        shared["wob"] = Wo_b.reshape(1, V)

    ii, pprt = np.meshgrid(np.arange(48), np.arange(128), indexing="xy")
    tblk = 2 * ii + pprt // 64
    ptgt = np.ascontiguousarray((tblk <= 46).astype(np.float32))

    in_maps = []
    for c in range(NC):
        xb = x_full[c * BC : (c + 1) * BC]          # [64, 48, 256]
        xt = np.ascontiguousarray(xb.transpose(2, 1, 0).reshape(H, NODES))
        xr = np.zeros((H, NODES), np.float32)
        xr[:, : NF * BC] = (
            xb[:, NF - 1 :: -1, :].transpose(2, 1, 0).reshape(H, NF * BC)
        )
        w2 = wid[c * BC : (c + 1) * BC]
        flat = np.ascontiguousarray(w2.T).reshape(-1)
        m = dict(shared)
        m["xt"] = xt.astype(wdt)
        m["xtr"] = xr.astype(wdt)
        m["tvt"] = np.ascontiguousarray(
            np.tile(tree_vec[c * BC : (c + 1) * BC].T, (1, 8))
        ).astype(wdt)
        m["qtgt"] = np.ascontiguousarray(flat.reshape(24, 128).T).astype(np.float32)
        m["ptgt"] = ptgt
        in_maps.append(m)
    return in_maps, wob_nonzero, float(Us_b.reshape(-1)[0])


def _combine(results, us_b):
    S = np.zeros(8, np.float64)
    for r in results:
        S += np.asarray(r["outp"], np.float64).sum(axis=0)
    pad_bce = max(us_b, 0.0) + np.log1p(np.exp(-abs(us_b)))
    pad_corr = 1.0 if us_b <= 0 else 0.0
    n_pad = NC * (PPAD - PROWS)
    p_loss = (S[0] - n_pad * pad_bce) / B
    p_acc = (S[1] - n_pad * pad_corr) / (PBLK * B)
    q_loss = (S[2] - S[3]) / B
    q_acc = S[4] / (QBLK * B)
    return np.array([q_loss, p_loss, q_acc, p_acc], np.float32)


def kernel(**inputs) -> np.ndarray:
    from concourse.bass_utils import run_bass_kernel_spmd

    in_maps, wob_nonzero, us_b = _prep_inputs(inputs)
    nc = _get_nc(wob_nonzero)
    res = run_bass_kernel_spmd(nc, in_maps, list(range(NC)))
    return _combine(res.results, us_b)

